# revision 12
# baseline (speedup 1.0000x reference)
"""Trainium2 Bass kernel for nn_DataExpander (dense_mlp), 8 NeuronCores.

Reference computation (B=512, G=20000, H=1024, E=512, O=2048):
    x_expanded  = lrelu(x @ W_ge.T + b_ge)                    [B, H]
    gene_emb    = lrelu(embedding_x @ W_em.T + b_em)          [G, H]
    weights     = softmax(x, axis=1)                          [B, G]
    weighted    = weights @ gene_emb                          [B, H]
    out         = lrelu(concat(x_expanded, weighted) @ W_c.T + b_c)   [B, O]

Sharding: the three big matmuls all contract over the gene axis (G=20000),
so each core takes a 2500-gene shard (padded to 2560 = 20 k-tiles):
  - phase A: partial pre-activation x_expanded.T sums (fp16 weights/x) +
    exp(x.T) written as fp8e4 into gene-block-PAIRED tiles,
  - den: partial softmax denominator via fp8 DoubleRow ones-matmuls over the
    exp pairs,
  - phase B: gene_emb rows for its genes — fp8e4 inputs, DoubleRow matmuls,
    bias-add on DVE, lrelu on ACT writing fp8e4 into gene-block-paired tiles,
  - phase C: partial softmax-numerator.T sums via fp8 DoubleRow.
Cross-core reduction: TWO AllReduces per rep (each collective op carries
~19us of fixed overhead on this fabric, so finer chunking serializes
worse): AR0 = [1025, 512] fp16 (pre.T + den row) fired right after
phase A + den, hidden behind B/C; AR1 = num.T [1024, 512] fp16 after
phase C — the only exposed collective in a single-shot launch.
A tiny warm-up AllReduce at kernel start absorbs the one-time cc-stream
barrier setup (~40us) and cross-core launch skew so AR0 doesn't pay it.

Phase D (combiner, output-feature tensor parallel) is split by AR
dependency: comb-left = lrelu(pre+b_ge) feeds W_c columns 0-1023 gated
only on AR0; the softmax-numerator half feeds W_c columns 1024-2047 RAW
(un-normalized), gated on AR1, and the 1/den normalization is applied
AFTER the matmul on DVE (psd_right * recip_bc, valid since den is
per-batch-column and the matmul is linear in columns) — this removes the
8 per-tile DVE muls and one fp16 rounding of the numerator path. The
numerator accumulation runs m-major so output tile 0 normalizes and
ships while tile 1 is still on the PE.
In multi-rep NEFFs phase D stays software-pipelined one rep behind (its
instructions are emitted right after the NEXT rep's AR0 fire point, when
every dependency is a full rep old, so the in-order PE queue never blocks
on a collective); the final rep's phase D runs at the end.

Weight-like tensors are loaded once and stay resident in SBUF; the
preamble DMAs are spread over the three DMA-capable engine queues (wge
even k-tiles + small consts on ACT's, x.T on SYNC, wge odd k-tiles +
embeddings + W_c on GPSIMD) so phase A streams without stalling instead
of queueing behind 8MB of weights on one queue.

Quantization budget (validated on HW: rel_max ~1.2e-3 vs the fp32
reference, 17x under the 2e-2 gate): fp8 errors in gene_emb/exp average
out through the softmax weighting; phase A and the combiner stay fp16
because dot-product relative error does NOT average down for those paths.

The walrus build in this container rejects instructions carrying more than
one sync-wait command, while TileContext emits multi-waits wherever deps
converge; _hoist_multi_waits rewrites those into single-wait engine nops.
"""
import sys

if '/opt/trn_rl_repo' not in sys.path:
    sys.path.insert(0, '/opt/trn_rl_repo')

import numpy as np

import concourse.bass as bass
import concourse.mybir as mybir
import concourse.tile as tile

N_CORES = 8
B = 512          # batch
G = 20000        # genes
GS = G // N_CORES            # 2500 genes per core
KT = 20                      # gene k-tiles per core
NP_ = KT // 2                # 10 gene-block pairs
GP = KT * 128                # 2560, padded gene shard
H = 1024         # hidden
E = 512          # embed
O = 2048         # output
OS = O // N_CORES            # 256 output rows per core

F32 = mybir.dt.float32
F16 = mybir.dt.float16
F8 = mybir.dt.float8e4
AF = mybir.ActivationFunctionType
DR = mybir.MatmulPerfMode.DoubleRow

NP8 = mybir.dt.np(F8)

_CACHE = {}


def _make_nop(nc, engine):
    bb = nc.main_func.blocks[-1]
    n_before = len(bb.instructions)
    nc.engines[engine].nop(nofuse=True)
    assert len(bb.instructions) == n_before + 1
    ins = bb.instructions[-1]
    bb.instructions = bb.instructions[:-1]
    return ins


def _hoist_multi_waits(nc, max_waits=1):
    total = 0
    for f in nc.m.functions:
        for bb in f.blocks:
            out = []
            changed = False
            for ins in bb.instructions:
                si = ins.sync_info
                if si is not None and len(si.on_wait) > max_waits:
                    waits = list(si.on_wait)
                    n_hoist = len(waits) - max_waits
                    for w in waits[:n_hoist]:
                        nop = _make_nop(nc, ins.engine)
                        nop.sync_info = mybir.SyncInfo(on_wait=[w], on_update=[])
                        out.append(nop)
                    ins.sync_info = mybir.SyncInfo(
                        on_wait=waits[n_hoist:], on_update=list(si.on_update)
                    )
                    changed = True
                    total += n_hoist
                out.append(ins)
            if changed:
                bb.instructions = out
    return total


def _build_nc(variant="full", reps=1):
    core_ids = list(range(N_CORES))
    nc = bass.Bass(target_bir_lowering=True)

    # weight-like parameters (resident in SBUF for the whole launch)
    wgeT = nc.declare_dram_parameter("wgeT", [KT, 128, H], F16, isOutput=False)
    emb8 = nc.declare_dram_parameter("emb8", [KT, 128, 2, 2, 128], F8, isOutput=False)
    wem8 = nc.declare_dram_parameter("wem8", [2, 128, 2, H], F8, isOutput=False)
    bemb = nc.declare_dram_parameter("bemb", [128, H], F32, isOutput=False)
    ones8 = nc.declare_dram_parameter("ones8", [128, 2, 16], F8, isOutput=False)
    ones1 = nc.declare_dram_parameter("ones1", [1, 128], F32, isOutput=False)
    bge = nc.declare_dram_parameter("bge", [H // 128, 128, 1], F32, isOutput=False)
    WcT = nc.declare_dram_parameter("WcT", [O // 128, 128, OS], F16, isOutput=False)
    bcc = nc.declare_dram_parameter("bcc", [OS // 128, 128, 1], F32, isOutput=False)
    # activations
    xT = nc.declare_dram_parameter("xT", [KT, 128, B], F16, isOutput=False)
    outT = nc.declare_dram_parameter("outT", [OS, B], F32, isOutput=True)

    with tile.TileContext(nc) as tc:
        with (
            tc.tile_pool(name="const", bufs=1) as const,
            tc.tile_pool(name="work", bufs=1) as work,
            tc.tile_pool(name="xt", bufs=6) as xt_p,
            tc.tile_pool(name="stage", bufs=3) as stage_p,
            tc.tile_pool(name="psum", bufs=8, space="PSUM") as psum,
            tc.tile_pool(name="dram", bufs=2, space="DRAM") as dram,
        ):
            # ---- resident weights, spread across engine DMA queues so the
            # first phase-A tiles land immediately ----
            # Preamble DMAs spread over the three DMA-capable queues so no
            # queue exceeds ~50GB/s while phase A streams:
            #   ACT:    bge, even wge k-tiles, bemb, small consts  (~3.2MB)
            #   SYNC:   x.T tiles + odd wge k-tiles (rep 0), staging (~5.2MB)
            #   GPSIMD: embeddings, W_em, W_c (slow SWDGE, B/D-time) (~2.9MB)
            bge_t = [const.tile([128, 1], F32, tag=f"bge{m}", name=f"bge{m}") for m in range(8)]
            for m in range(8):
                nc.scalar.dma_start(out=bge_t[m][:], in_=bge[m])
            wge_t = [const.tile([128, H], F16, tag=f"wge{k}", name=f"wge{k}") for k in range(KT)]
            for k in range(0, KT, 2):
                nc.scalar.dma_start(out=wge_t[k][:], in_=wgeT[k])
            bemb_t = const.tile([128, H], F32, tag="bemb")
            nc.scalar.dma_start(out=bemb_t[:], in_=bemb[:])
            ch8 = [const.tile([128, 2, 2, 128], F8, tag=f"ch{g}", name=f"ch{g}") for g in range(KT)]
            for g in range(KT):
                nc.gpsimd.dma_start(out=ch8[g][:], in_=emb8[g])
            wem_t = [const.tile([128, 2, H], F8, tag=f"wem{kp}", name=f"wem{kp}") for kp in range(2)]
            for kp in range(2):
                nc.gpsimd.dma_start(out=wem_t[kp][:], in_=wem8[kp])
            ones8_t = const.tile([128, 2, 16], F8, tag="ones8")
            nc.scalar.dma_start(out=ones8_t[:], in_=ones8[:])
            ones1_t = const.tile([1, 128], F32, tag="ones1")
            nc.scalar.dma_start(out=ones1_t[:], in_=ones1[:])
            bcc_t = [const.tile([128, 1], F32, tag=f"bcc{m}", name=f"bcc{m}") for m in range(2)]
            for m in range(2):
                nc.scalar.dma_start(out=bcc_t[m][:], in_=bcc[m])
            wc_t = [const.tile([128, OS], F16, tag=f"wc{k}", name=f"wc{k}") for k in range(16)]
            for k in range(16):
                nc.gpsimd.dma_start(out=wc_t[k][:], in_=WcT[k])

            # Warm-up collective LAST on the gpsimd queue (it executes at the
            # cc-stream's ~65us init floor no matter how early it is issued,
            # so issuing it after the weight DMAs costs nothing and keeps the
            # queue flowing; only AR0, which can't start earlier anyway, sits
            # behind it). It absorbs the cc-stream barrier setup + skew.
            if variant == "full":
                db = dram.tile([1, 1], F32, tag="dbar")
                db_o = dram.tile([1, 1], F32, addr_space="Shared", tag="dbaro")
                nc.gpsimd.dma_start(out=db[:], in_=bge[0][:1, :])
                nc.gpsimd.collective_compute(
                    "AllReduce", mybir.AluOpType.add,
                    replica_groups=[core_ids],
                    ins=[db.opt()], outs=[db_o.opt()],
                )

            # ---- persistent work tiles (reused every rep; WAR deps order reps) ----
            ge8 = [work.tile([128, 2, H], F8, tag=f"ge{t}", name=f"ge{t}") for t in range(NP_)]
            ex8 = [work.tile([128, 2, B], F8, tag=f"ex{t}", name=f"ex{t}") for t in range(NP_)]
            st_pre = work.tile([128, 8, 512], F16, tag="stpre")
            st_num = work.tile([128, 8, 512], F16, tag="stnum")
            rt_pre = work.tile([128, 8, 512], F16, tag="rtpre")
            rt_num = work.tile([128, 8, 512], F16, tag="rtnum")
            comb_l = [work.tile([128, B], F16, tag=f"cb{k}", name=f"cb{k}") for k in range(8)]
            den_sb = work.tile([1, B], F16, tag="den")
            recip = work.tile([1, B], F32, tag="recip")
            recip_bc = work.tile([128, B], F32, tag="recipbc")
            s_mul = [work.tile([128, 512], F32, tag=f"smul{m}", name=f"smul{m}") for m in range(2)]
            s_add = [work.tile([128, 512], F32, tag=f"sadd{m}", name=f"sadd{m}") for m in range(2)]
            ot = work.tile([128, 2, 512], F32, tag="ot")

            def phase_d(ctx):
                b_pre_o, b_num_o = ctx
                # den row -> reciprocal -> broadcast to 128 partitions (1 PE slot)
                nc.sync.dma_start(out=den_sb[:], in_=b_pre_o[H:H + 1, :])
                nc.vector.reciprocal(recip[:], den_sb[:])
                ps_bc = psum.tile([128, 512], F32, tag="acc")
                nc.tensor.matmul(ps_bc[:], ones1_t[:], recip[:], start=True, stop=True)
                nc.vector.tensor_copy(recip_bc[:], ps_bc[:])

                # read back AllReduced pre.T / num.T
                nc.sync.dma_start(
                    out=rt_pre[:],
                    in_=b_pre_o[:H, :].rearrange("(m p) b -> p m b", p=128))
                nc.sync.dma_start(
                    out=rt_num[:],
                    in_=b_num_o[:].rearrange("(m p) b -> p m b", p=128))
                # left half of comb: lrelu(pre + b_ge) (fp16 moving tiles)
                for k in range(8):
                    nc.scalar.activation(
                        comb_l[k][:], rt_pre[:, k, :], AF.Lrelu,
                        bias=bge_t[k][:], scale=1.0, alpha=0.01,
                    )
                psd_l = [psum.tile([128, 512], F32, tag="acc", name=f"psdl{m}") for m in range(2)]
                psd_r = [psum.tile([128, 512], F32, tag="acc", name=f"psdr{m}") for m in range(2)]
                # AR0-gated accumulation (W_c columns 0..1023)
                for k in range(8):
                    for m in range(OS // 128):
                        nc.tensor.matmul(
                            psd_l[m][:], wc_t[k][:, bass.ts(m, 128)], comb_l[k][:],
                            start=(k == 0), stop=(k == 7),
                        )
                # AR1-gated accumulation (W_c columns 1024..2047) with the raw
                # numerator as moving operand, m-major so output tile m=0
                # normalizes and ships while m=1 is still on the PE
                for m in range(OS // 128):
                    for k in range(8):
                        nc.tensor.matmul(
                            psd_r[m][:], wc_t[8 + k][:, bass.ts(m, 128)], rt_num[:, k, :],
                            start=(k == 0), stop=(k == 7),
                        )
                    # normalize the numerator half after the matmul (valid:
                    # den is per-batch-column), combine with the left half,
                    # bias+lrelu, and ship this output tile
                    nc.vector.tensor_mul(s_mul[m][:], psd_r[m][:], recip_bc[:])
                    nc.vector.tensor_add(s_add[m][:], s_mul[m][:], psd_l[m][:])
                    nc.scalar.activation(
                        ot[:, m, :], s_add[m][:], AF.Lrelu,
                        bias=bcc_t[m][:], scale=1.0, alpha=0.01,
                    )
                    nc.sync.dma_start(
                        out=outT[:].rearrange("(m p) b -> p m b", p=128)[:, m, :],
                        in_=ot[:, m, :])

            pending_d = None
            for _rep in range(reps):
                b_pre = dram.tile([H + 1, B], F16, tag="bpre", name=f"bpre{_rep}")
                b_num = dram.tile([H, B], F16, tag="bnum", name=f"bnum{_rep}")
                b_pre_o = dram.tile([H + 1, B], F16, addr_space="Shared", tag="bpreo", name=f"bpreo{_rep}")
                b_num_o = dram.tile([H, B], F16, addr_space="Shared", tag="bnumo", name=f"bnumo{_rep}")

                # ---- phase A: pre_ge.T partials (fp16) + exp -> fp8 pairs ----
                ps_ge = [psum.tile([128, 512], F32, tag="acc", name=f"psge{m}") for m in range(8)]
                for k in range(KT):
                    if _rep == 0 and k % 2 == 0 and k + 1 < KT:
                        # odd wge tiles ride SYNC's fast HWDGE queue, paced
                        # just ahead of the x tiles that need them
                        nc.sync.dma_start(out=wge_t[k + 1][:], in_=wgeT[k + 1])
                    xt = xt_p.tile([128, B], F16, tag="xt")
                    nc.sync.dma_start(out=xt[:], in_=xT[k])
                    for m in range(8):
                        nc.tensor.matmul(
                            ps_ge[m][:], wge_t[k][:, bass.ts(m, 128)], xt[:],
                            start=(k == 0), stop=(k == KT - 1),
                        )
                    nc.scalar.activation(ex8[k // 2][:, k % 2, :], xt[:], AF.Exp)

                # drain pre.T into one fp16 staging tile, 1 DMA to b_pre
                for m in range(8):
                    nc.vector.tensor_copy(st_pre[:, m, :], ps_ge[m][:])
                nc.sync.dma_start(
                    out=b_pre[:H, :].rearrange("(m p) b -> p m b", p=128),
                    in_=st_pre[:],
                )

                # ---- den partial: fp8 DoubleRow ones-matmuls over exp pairs ----
                ps_den = psum.tile([1, 512], F32, tag="acc")
                for t in range(NP_):
                    nc.tensor.matmul(
                        ps_den[:], ones8_t[:, :, :1], ex8[t][:],
                        start=(t == 0), stop=(t == NP_ - 1), perf_mode=DR,
                    )
                st_den = stage_p.tile([1, 512], F16, tag="stden")
                nc.vector.tensor_copy(st_den[:], ps_den[:])
                nc.sync.dma_start(out=b_pre[H:H + 1, :], in_=st_den[:])

                if variant == "full":
                    nc.gpsimd.collective_compute(
                        "AllReduce", mybir.AluOpType.add,
                        replica_groups=[core_ids],
                        ins=[b_pre.opt()], outs=[b_pre_o.opt()],
                    )
                else:
                    b_pre_o, b_num_o = b_pre, b_num

                # ---- phase D of the PREVIOUS rep: every dependency is a full
                # rep old here, so the in-order PE queue never stalls ----
                if pending_d is not None:
                    phase_d(pending_d)

                # ---- phase B: gene_emb = lrelu(embT.T @ WemT + b_em) -> fp8 pairs ----
                for g in range(KT):
                    for n in range(2):
                        ps = psum.tile([128, 512], F32, tag="acc")
                        for kp in range(2):
                            nc.tensor.matmul(
                                ps[:], ch8[g][:, kp, :, :],
                                wem_t[kp][:, :, bass.ts(n, 512)],
                                start=(kp == 0), stop=(kp == 1), perf_mode=DR,
                            )
                        st = stage_p.tile([128, 512], F32, tag="stageb")
                        nc.vector.tensor_add(st[:], ps[:], bemb_t[:, bass.ts(n, 512)])
                        nc.scalar.activation(
                            ge8[g // 2][:, g % 2, bass.ts(n, 512)], st[:], AF.Lrelu,
                            bias=0.0, scale=1.0, alpha=0.01,
                        )

                # ---- phase C: numerator.T partials via fp8 DoubleRow ----
                for m in range(8):
                    ps = psum.tile([128, 512], F32, tag="acc")
                    for t in range(NP_):
                        nc.tensor.matmul(
                            ps[:], ge8[t][:, :, bass.ts(m, 128)], ex8[t][:],
                            start=(t == 0), stop=(t == NP_ - 1), perf_mode=DR,
                        )
                    nc.vector.tensor_copy(st_num[:, m, :], ps[:])
                nc.sync.dma_start(
                    out=b_num[:].rearrange("(m p) b -> p m b", p=128),
                    in_=st_num[:],
                )
                if variant == "full":
                    nc.gpsimd.collective_compute(
                        "AllReduce", mybir.AluOpType.add,
                        replica_groups=[core_ids],
                        ins=[b_num.opt()], outs=[b_num_o.opt()],
                    )

                pending_d = (b_pre_o, b_num_o)

            phase_d(pending_d)

    _hoist_multi_waits(nc)
    return nc


def _prep_inputs(x, embedding_x, W_ge, b_ge, W_em, b_em, W_c, b_c):
    """Build per-core input maps (hardcoded sharding + quantization)."""
    x = np.ascontiguousarray(x, dtype=np.float32)
    xT16 = x.T.astype(np.float16)                      # [G, B]
    WgeT16 = np.asarray(W_ge, np.float32).T.astype(np.float16)  # [G, H]
    bemb_np = np.tile(np.asarray(b_em, np.float32).reshape(1, H), (128, 1))
    # wem8[kp][p, s, h] = W_em.T[256kp + 128s + p, h]
    WemT8 = np.asarray(W_em, np.float32).T.astype(NP8)          # [E, H]
    wem8_np = WemT8.reshape(2, 2, 128, H).transpose(0, 2, 1, 3).copy()
    ones8_np = np.zeros((128, 2, 16), NP8)
    ones8_np[:, :, 0] = 1.0
    ones1_np = np.ones((1, 128), np.float32)
    bge_np = np.asarray(b_ge, np.float32).reshape(H // 128, 128, 1)
    bcc_full = np.asarray(b_c, np.float32)
    WcT_full = np.asarray(W_c, np.float32).T.astype(np.float16)  # [2H, O]
    E8 = np.asarray(embedding_x, np.float32).astype(NP8)         # [G, E]

    in_maps = []
    for c in range(N_CORES):
        sl = slice(GS * c, GS * (c + 1))
        # xT: [KT, 128, B]; pad genes get -1e4 so exp underflows to 0
        xT_c = np.full((GP, B), -1e4, np.float16)
        xT_c[:GS] = xT16[sl]
        wge_c = np.zeros((GP, H), np.float16)
        wge_c[:GS] = WgeT16[sl]
        # emb8[g][p, kp, s, j] = E.T[e = 256kp+128s+p, local gene 128g + j]
        embT_c = np.zeros((E, GP), NP8)
        embT_c[:, :GS] = E8[sl].T
        emb8_c = (
            embT_c.reshape(2, 2, 128, KT, 128)   # [kp, s, p, g, j]
            .transpose(3, 2, 0, 1, 4)            # [g, p, kp, s, j]
            .copy()
        )
        WcT_c = np.ascontiguousarray(
            WcT_full[:, OS * c:OS * (c + 1)]
        ).reshape(O // 128, 128, OS)
        bcc_c = bcc_full[OS * c:OS * (c + 1)].reshape(OS // 128, 128, 1)
        in_maps.append({
            "wgeT": wge_c.reshape(KT, 128, H),
            "emb8": emb8_c,
            "wem8": wem8_np,
            "bemb": bemb_np,
            "ones8": ones8_np,
            "ones1": ones1_np,
            "bge": bge_np,
            "WcT": WcT_c,
            "bcc": bcc_c,
            "xT": xT_c.reshape(KT, 128, B),
        })
    return in_maps


def _get_runner(variant="full", reps=1):
    """Build (once) a cached jitted 8-core runner following bass2jax's
    run_bass_via_pjrt shard_map recipe, so repeated calls don't re-trace."""
    key = ("runner", variant, reps)
    if key in _CACHE:
        return _CACHE[key]

    import jax
    from jax.sharding import Mesh, PartitionSpec
    try:
        from jax.experimental.shard_map import shard_map
    except ImportError:
        from jax.shard_map import shard_map
    from concourse import bass2jax

    bass2jax.install_neuronx_cc_hook()
    nc = _build_nc(variant, reps)

    partition_name = (
        nc.partition_id_tensor.name if nc.partition_id_tensor else None
    )
    in_names = []
    out_names = []
    out_avals = []
    zero_outs = []
    for alloc in nc.m.functions[0].allocations:
        if not isinstance(alloc, mybir.MemoryLocationSet):
            continue
        name = alloc.memorylocations[0].name
        if alloc.kind == "ExternalInput":
            if name != partition_name:
                in_names.append(name)
        elif alloc.kind == "ExternalOutput":
            out_names.append(name)
            shape = tuple(alloc.tensor_shape)
            dtype = mybir.dt.np(alloc.dtype)
            out_avals.append(jax.core.ShapedArray(shape, dtype))
            zero_outs.append(np.zeros(shape, dtype))
    n_params = len(in_names)
    all_in_names = in_names + out_names
    if partition_name is not None:
        all_in_names = all_in_names + [partition_name]

    def _body(*args):
        operands = list(args)
        if partition_name is not None:
            operands.append(bass2jax.partition_id_tensor())
        outs = bass2jax._bass_exec_p.bind(
            *operands,
            out_avals=tuple(out_avals),
            in_names=tuple(all_in_names),
            out_names=tuple(out_names),
            lowering_input_output_aliases=(),
            sim_require_finite=True,
            sim_require_nnan=True,
            nc=nc,
        )
        return tuple(outs)

    devices = jax.devices()[:N_CORES]
    mesh = Mesh(np.asarray(devices), ("core",))
    n_outs = len(out_names)
    sharded = jax.jit(
        shard_map(
            _body,
            mesh=mesh,
            in_specs=(PartitionSpec("core"),) * (n_params + n_outs),
            out_specs=(PartitionSpec("core"),) * n_outs,
            check_rep=False,
        ),
        keep_unused=True,
    )
    runner = {
        "fn": sharded,
        "in_names": in_names,
        "out_names": out_names,
        "zero_outs": zero_outs,
        "mesh": mesh,
    }
    _CACHE[key] = runner
    return runner


def _run(in_maps):
    r = _get_runner()
    concat_in = [
        np.concatenate([in_maps[c][name] for c in range(N_CORES)], axis=0)
        for name in r["in_names"]
    ]
    concat_zeros = [
        np.zeros((N_CORES * z.shape[0], *z.shape[1:]), z.dtype)
        for z in r["zero_outs"]
    ]
    out_arrs = r["fn"](*concat_in, *concat_zeros)
    out_idx = r["out_names"].index("outT")
    outT_all = np.asarray(out_arrs[out_idx]).reshape(N_CORES, OS, B)
    return outT_all


def kernel(x, embedding_x, W_ge, b_ge, W_em, b_em, W_c, b_c):
    in_maps = _prep_inputs(x, embedding_x, W_ge, b_ge, W_em, b_em, W_c, b_c)
    outT_all = _run(in_maps)
    # outT_all[c] is rows [OS*c : OS*(c+1)] of out.T -> assemble and transpose
    out_T = outT_all.reshape(O, B)
    return np.ascontiguousarray(out_T.T)
